# revision 14
# baseline (speedup 1.0000x reference)
"""Distributed ring-attention kernel for Trainium2 (8 NeuronCores, Bass/Tile).

Strategy (seq-parallel attention, full softmax without max-subtraction):
  - Host: transpose/cast inputs to bf16; shard x.T column-wise (seq) across 8 cores.
  - Per core: project Q/K/V for its 512-seq shard; AllGather K^T and V
    across cores in fp8e4m3 (halves collective bytes vs bf16; V carries a
    baked-in ones column per head so the softmax denominator rides the PV
    matmul); compute full attention for its Q shard over the whole
    4096-length K/V; out-projection; write its y shard.
  - Scores are computed transposed (S^T = K @ Q^T, kpos on partitions) so the
    exp'd probabilities feed the P@V matmul directly as the stationary-side
    contraction. Softmax skips max-subtraction: scores are O(1) here.
  - exp is split between the scalar engine (table exp) and the vector engine
    (one-op int16 Schraudolph producing bf16 bit patterns) so neither engine
    is the serial bottleneck.
"""

import numpy as np
import ml_dtypes

HID = 1024
HEADS = 16
HD = 64
S = 4096
NCORES = 8
SQ = S // NCORES          # 512 q rows per core
PAIRS = HEADS // 2        # 8 head pairs (128 rows of qkvT per pair)
KTILES = S // 128         # 32 kpos tiles per head
VAUG = HD + 1             # 65: V plus ones column
SCALE = 1.0 / np.sqrt(HD)

# ---- tuning dials ----
SCHRAU_MOD = 2            # every SCHRAU_MOD'th exp group runs on DVE (0=off)
SCHRAU_A = SCALE * 128.0 / np.log(2.0)
SCHRAU_B = 127.0 * 128.0 - 7.5   # mixed-bias-matched C=+7.5

_cache = {}


def _build():
    import concourse.bass as bass
    import concourse.mybir as mybir
    import concourse.tile as tile
    from concourse import bacc

    dt = mybir.dt
    nc = bacc.Bacc("TRN2", target_bir_lowering=False, debug=False,
                   num_devices=NCORES)

    xT = nc.dram_tensor("xT", [HID, SQ], dt.bfloat16, kind="ExternalInput").ap()
    wqkvT = nc.dram_tensor("wqkvT", [HID, 3 * HID], dt.bfloat16,
                           kind="ExternalInput").ap()
    woutT = nc.dram_tensor("woutT", [HID, HID], dt.bfloat16,
                           kind="ExternalInput").ap()
    y = nc.dram_tensor("y", [SQ, HID], dt.float32, kind="ExternalOutput").ap()

    with tile.TileContext(nc) as tc:
        _body(nc, tc, bass, mybir, xT, wqkvT, woutT, y)

    nc.compile()
    return nc


def _body(nc, tc, bass, mybir, xT, wqkvT, woutT, y):
    dt = mybir.dt
    f32, bf16, f8 = dt.float32, dt.bfloat16, dt.float8e4
    i16 = dt.int16
    RG = [list(range(NCORES))]

    with (
        tc.tile_pool(name="dram", bufs=1, space="DRAM") as dram,
        tc.tile_pool(name="resident", bufs=1) as res,
        tc.tile_pool(name="stream", bufs=1) as st,
    ):
        # ---- DRAM bounce buffers for collectives (fp8 payloads), one set
        # per UNIT of head pairs ----
        UNITS = [[0, 1], [2, 3], [4, 5], [6, 7]]
        unit_of = {}
        for u, prs in enumerate(UNITS):
            for i, p in enumerate(prs):
                unit_of[p] = (u, i)
        # KT and V each gathered in two 4-pair halves: few enough ops that
        # the ~10us-per-op collective overhead stays off the critical path,
        # small enough that pair 0 starts early
        ktbh = [dram.tile([4 * 128, SQ], f8, name=f"ktb{h}")
                for h in range(2)]
        ktgh = [dram.tile([NCORES * 4 * 128, SQ], f8,
                          addr_space="Shared", name=f"ktg{h}")
                for h in range(2)]
        vbh = [dram.tile([SQ, 8 * VAUG], bf16, name=f"vb{h}")
               for h in range(2)]
        vgh = [dram.tile([S, 8 * VAUG], bf16, addr_space="Shared",
                         name=f"vg{h}") for h in range(2)]

        # ---- load xT (hidden x local-seq), 8 resident tiles ----
        xt = []
        for k in range(8):
            t = res.tile([128, SQ], bf16, tag=f"xt{k}", name=f"xt{k}")
            nc.sync.dma_start(t[:], xT[k * 128:(k + 1) * 128, :])
            xt.append(t)

        # wqkvT strip views for batched weight loads
        wq4 = wqkvT.rearrange("(k p) (m c) -> p m k c", p=128, c=128)

        def kt_proj(m, psP):
            """K^T rows for pair m (qkvT rows 1024+m*128)."""
            ws = st.tile([128, 8 * 128], bf16, tag="wl", bufs=4)
            nc.sync.dma_start(ws.rearrange("p (k c) -> p k c", c=128),
                              wq4[:, 8 + m, :, :])
            ps = psP.tile([128, SQ], f32, tag="proj", bufs=4)
            for k in range(8):
                nc.tensor.matmul(ps[:], ws[:, k * 128:(k + 1) * 128],
                                 xt[k][:], start=(k == 0), stop=(k == 7))
            sb = st.tile([128, SQ], f8, tag="kt_stage", bufs=4)
            nc.vector.tensor_copy(sb[:], ps[:])
            nc.sync.dma_start(ktbh[m // 4][(m % 4) * 128:(m % 4 + 1) * 128, :],
                              sb[:])
            if m % 4 == 3:
                nc.gpsimd.collective_compute(
                    "AllGather", mybir.AluOpType.bypass, replica_groups=RG,
                    ins=[ktbh[m // 4].opt()], outs=[ktgh[m // 4].opt()])

        wv2 = wqkvT.rearrange("(k p) (m c) -> p m k c", p=128, c=128)

        # staging tiles for V with baked ones columns: pre-memset to 1.0 so
        # col 64 of each 65-wide head block stays 1.0 forever
        vst_bufs = 4
        for _ in range(vst_bufs):
            vst = st.tile([128, 4 * VAUG], bf16, tag="vst", bufs=vst_bufs)
            nc.vector.memset(vst[:], 1.0)

        def v_proj(u, psP):
            """V rows (natural [s, head-blocks-of-65]) for unit u -> vb + AG."""
            prs = UNITS[u]
            n = len(prs)
            wvs = st.tile([128, 8 * n * 128], bf16, tag="wvs", bufs=3)
            wvs3 = wvs.rearrange("p (k c) -> p k c", c=n * 128)
            nc.sync.dma_start(
                wvs3.rearrange("p k (pr c) -> p k pr c", c=128),
                wv2[:, 16 + prs[0]:16 + prs[0] + n, :, :].rearrange(
                    "p pr k c -> p k pr c"))
            for sti in range(4):
                ps = psP.tile([128, n * 128], f32, tag="proj", bufs=4)
                for k in range(8):
                    nc.tensor.matmul(
                        ps[:], xt[k][:, sti * 128:(sti + 1) * 128],
                        wvs[:, k * n * 128:(k + 1) * n * 128],
                        start=(k == 0), stop=(k == 7))
                vst = st.tile([128, 4 * VAUG], bf16, tag="vst", bufs=vst_bufs)
                nc.vector.tensor_copy(
                    vst.rearrange("p (hh c) -> p hh c", c=VAUG)[:, :, 0:HD],
                    ps.rearrange("p (hh c) -> p hh c", c=HD))
                nc.sync.dma_start(
                    vbh[u // 2][sti * 128:(sti + 1) * 128,
                                (u % 2) * 4 * VAUG:(u % 2 + 1) * 4 * VAUG],
                    vst[:])
            if u % 2 == 1:
                nc.gpsimd.collective_compute(
                    "AllGather", mybir.AluOpType.bypass, replica_groups=RG,
                    ins=[vbh[u // 2].opt()], outs=[vgh[u // 2].opt()])

        qt = [None] * PAIRS

        def q_proj(m, psP):
            ws = st.tile([128, 8 * 128], bf16, tag="wl", bufs=4)
            nc.sync.dma_start(ws.rearrange("p (k c) -> p k c", c=128),
                              wq4[:, m, :, :])
            ps = psP.tile([128, SQ], f32, tag="proj", bufs=4)
            for k in range(8):
                nc.tensor.matmul(ps[:], ws[:, k * 128:(k + 1) * 128],
                                 xt[k][:], start=(k == 0), stop=(k == 7))
            t = res.tile([128, SQ], bf16, tag=f"qt{m}", name=f"qt{m}")
            nc.vector.tensor_copy(t[:], ps[:])
            qt[m] = t

        with tc.tile_pool(name="psP", bufs=1, space="PSUM") as psP:
            # emit each unit's kT then V so the collectives fire in exactly
            # the order attention consumes them
            for m in range(4):
                kt_proj(m, psP)
            v_proj(0, psP)
            v_proj(1, psP)
            q_proj(0, psP)
            for m in range(4, PAIRS):
                kt_proj(m, psP)
            v_proj(2, psP)
            v_proj(3, psP)
            for m in range(1, PAIRS):
                q_proj(m, psP)

        # ---- attention (head pairs row-packed on the PE array) ----
        attn = []
        for p in range(PAIRS):
            t = res.tile([128, SQ], bf16, tag=f"attn{p}", name=f"attn{p}")
            attn.append(t)

        wo5 = woutT.rearrange("(pp r) (o c) -> r o pp c", r=128, c=512)
        wo = []
        for och in range(2):
            w = res.tile([128, PAIRS * 512], bf16, tag=f"wo{och}",
                         name=f"wo{och}")
            nc.sync.dma_start(
                w.rearrange("r (pp c) -> r pp c", c=512), wo5[:, och])
            wo.append(w)
        with tc.tile_pool(name="psA", bufs=1, space="PSUM") as psA:
            for p in range(PAIRS):
                # pair K^T strip [128, 4096] fp8: rows 0..63 head 2p,
                # 64..127 head 2p+1
                ktg3 = ktgh[p // 4].rearrange("(c i r) q -> i r c q",
                                              i=4, r=128)
                kth = st.tile([128, S], f8, tag="kth", bufs=3)
                nc.gpsimd.dma_start(
                    kth.rearrange("r (c q) -> r c q", q=SQ), ktg3[p % 4])
                # V with ones columns, already baked by the sender
                vg3 = vgh[p // 4].rearrange(
                    "(t q) (i hh c) -> i hh q t c", q=128,
                    i=4, c=VAUG)[p % 4]
                vah = []
                for e in range(2):
                    va = st.tile([128, KTILES * VAUG], bf16, tag="vah",
                                 bufs=6)
                    nc.gpsimd.dma_start(
                        va.rearrange("q (t c) -> q t c", c=VAUG), vg3[e])
                    vah.append(va)

                pv = [psA.tile([128, 512], f32, tag="pv", bufs=2,
                               name=f"pv{p}_{e}") for e in range(2)]

                # slot stream: (t, even), (t, odd) pairs; exp groups of 2.
                RA = 15
                slots = [(t, e) for t in range(KTILES) for e in range(2)]
                groups = [slots[gs:gs + 2]
                          for gs in range(0, len(slots), 2)]
                pts = []

                def emit_scores(group, gi):
                    gw = 512 * len(group)
                    sc = psA.tile([128, 1024], f32, tag="sc", bufs=3)
                    for idx, (t, e) in enumerate(group):
                        nc.tensor.matmul(
                            sc[:, idx * 512:(idx + 1) * 512],
                            kth[e * 64:(e + 1) * 64, t * 128:(t + 1) * 128],
                            qt[p][e * 64:(e + 1) * 64, :],
                            start=True, stop=True,
                            tile_position=(e * 64, 0))
                    pt = st.tile([128, 1024], bf16, tag="pt", bufs=RA + 2)
                    if SCHRAU_MOD and gi % SCHRAU_MOD == 0:
                        # one-op int16 Schraudolph: bits land as bf16 exp
                        nc.vector.tensor_scalar(
                            pt[:, 0:gw].bitcast(i16), sc[:, 0:gw],
                            float(SCHRAU_A), float(SCHRAU_B),
                            mybir.AluOpType.mult, mybir.AluOpType.add)
                    else:
                        nc.scalar.activation(
                            pt[:, 0:gw], sc[:, 0:gw],
                            mybir.ActivationFunctionType.Exp,
                            scale=float(SCALE))
                    pts.append(pt)

                def emit_pv(group, pt):
                    for idx, (t, e) in enumerate(group):
                        nc.tensor.matmul(
                            pv[e][0:VAUG, :],
                            vah[e][:, t * VAUG:(t + 1) * VAUG],
                            pt[:, idx * 512:(idx + 1) * 512],
                            start=(t == 0), stop=(t == KTILES - 1))

                for gi, group in enumerate(groups):
                    emit_scores(group, gi)
                    if gi >= RA:
                        emit_pv(groups[gi - RA], pts[gi - RA])
                for gi in range(len(groups) - RA, len(groups)):
                    emit_pv(groups[gi], pts[gi])

                # normalize: out_head = pv_data / l  (l = ones-column row 64).
                for e in range(2):
                    pvs = st.tile([VAUG, 512], f32, tag="pvs", bufs=6)
                    nc.vector.tensor_copy(pvs[:], pv[e][0:VAUG, :])
                    l0 = st.tile([1, 512], f32, tag="l0", bufs=2)
                    nc.sync.dma_start(l0[:], pvs[64:65, :])
                    lb = st.tile([64, 512], f32, tag="lb", bufs=2)
                    nc.gpsimd.partition_broadcast(lb[:], l0[:])
                    rb = st.tile([64, 512], f32, tag="rb", bufs=2)
                    nc.vector.reciprocal_approx_fast(rb[:], lb[:])
                    if e == 0:
                        nc.vector.tensor_mul(attn[p][0:64, :],
                                             pvs[0:64, :], rb[:])
                    else:
                        ao = st.tile([64, SQ], bf16, tag="ao", bufs=2)
                        nc.vector.tensor_mul(ao[:], pvs[0:64, :], rb[:])
                        nc.gpsimd.dma_start(attn[p][64:128, :], ao[:])

        # ---- out projection ----
        with tc.tile_pool(name="psY", bufs=1, space="PSUM") as psY:
            for sti in range(4):
                for och in range(2):
                    psa = psY.tile([128, 512], f32, tag="ya", bufs=8)
                    for p in range(PAIRS):
                        nc.tensor.matmul(
                            psa[:], attn[p][:, sti * 128:(sti + 1) * 128],
                            wo[och][:, p * 512:(p + 1) * 512],
                            start=(p == 0), stop=(p == PAIRS - 1))
                    ysb = st.tile([128, 512], f32, tag="ysb", bufs=4)
                    nc.vector.tensor_copy(ysb[:], psa[:])
                    nc.sync.dma_start(
                        y[sti * 128:(sti + 1) * 128,
                          och * 512:(och + 1) * 512], ysb[:])


def _get_nc():
    if "nc" not in _cache:
        _cache["nc"] = _build()
    return _cache["nc"]


def kernel(x, W_qkv, W_out, _trace=False):
    from concourse.bass_utils import run_bass_kernel_spmd

    nc = _get_nc()
    bf16 = ml_dtypes.bfloat16

    x = np.asarray(x)
    xTf = np.ascontiguousarray(x.reshape(S, HID).T).astype(bf16)   # [HID, S]
    wqkvT = np.ascontiguousarray(np.asarray(W_qkv).T).astype(bf16)
    woutT = np.ascontiguousarray(np.asarray(W_out).T).astype(bf16)

    in_maps = []
    for c in range(NCORES):
        in_maps.append({
            "xT": np.ascontiguousarray(xTf[:, c * SQ:(c + 1) * SQ]),
            "wqkvT": wqkvT,
            "woutT": woutT,
        })
    res = run_bass_kernel_spmd(nc, in_maps, core_ids=list(range(NCORES)),
                               trace=_trace)
    out = np.concatenate([res.results[c]["y"] for c in range(NCORES)],
                         axis=0)
    out = out.reshape(1, S, HID).astype(np.float32)
    if _trace:
        kernel.last_results = res
    return out
